# revision 16
# baseline (speedup 1.0000x reference)
"""Multi-head attention block (QKV linear -> softmax attention -> proj linear)
for Trainium2, SPMD over 8 NeuronCores.

Sharding: 8 shards = batch (4) x head-group (2 groups of 6 heads).
Each core computes, for its (b, g):
    qkv   = x[b] @ Wqkv[:, cols(g)]            (bf16 matmul, fp32 accum)
    S^T_h = K_h Q_h^T   per head               (keys on partitions)
    P^T_h = exp(SCALE * S^T_h)                 (ACT engine, bf16 out)
    out_h = (P_h @ [V_h | 1]) -> normalize rows by the ones-column sum
    y_g   = concat_h(out_h) @ Wproj[rows(g), :]    (partial, fp32 out)
Host sums the two head-group partials per batch and adds bproj.

Head-PAIR emission: heads 2j and 2j+1 have their Q/K channels on SBUF
partitions 0-63 / 64-127 of the same qkT block, so their K=64 score
matmuls carry PE tile positions (0,0) / (64,0) and run CONCURRENTLY in
the PE array when issued back-to-back (measured 3x vs serial).  Scores
are emitted in n-quarters (512 cols); each (mt, quarter) fills a
[128,1024] psum pair-chunk ([head 2j | head 2j+1]) consumed by one
1024-wide exp ACTIVATE into a packed pts tile [128, mt-parity, head,
512].  PV/transposes/proj interleave as work items one quarter behind.

Shapes hardcoded: x [4, 2048, 768], Wqkv [768, 2304], Wproj [768, 768].
"""

import os
from contextlib import ExitStack

import numpy as np
import ml_dtypes

import concourse.bass as bass
import concourse.mybir as mybir
import concourse.tile as tile
from concourse import bacc
from concourse.bass_utils import run_bass_kernel_spmd
from concourse.masks import make_identity

B, N, C = 4, 2048, 768
H, D = 12, 64          # total heads, head dim
G = 2                  # head groups (tensor-parallel axis)
HL = H // G            # heads per core = 6
NP = HL // 2           # head pairs per core = 3
SCALE = D ** -0.5
P = 128
CB = C // P            # 6 contraction blocks
NT = N // P            # 16 row tiles
EG = HL * D            # 384 = per-group width of Q / K / V
NCORES = 8

f32 = mybir.dt.float32
bf16 = mybir.dt.bfloat16

PT_BUFS = int(os.environ.get("KRN_PT_BUFS", "24"))


def _build_program():
    nc = bacc.Bacc("TRN2", target_bir_lowering=False, debug=False)

    xT = nc.dram_tensor("xT", [C, N], bf16, kind="ExternalInput")           # x[b].T
    wqkv = nc.dram_tensor("wqkv", [C, 3 * EG], bf16, kind="ExternalInput")  # [Qg|Kg|Vg]
    wproj = nc.dram_tensor("wproj", [EG, C], bf16, kind="ExternalInput")    # group rows
    y = nc.dram_tensor("y", [N, C], bf16, kind="ExternalOutput")            # partial out

    with tile.TileContext(nc) as tc, ExitStack() as ctx:
        persist = ctx.enter_context(tc.tile_pool(name="persist", bufs=1))
        ptpool = ctx.enter_context(tc.tile_pool(name="ptpool", bufs=1))
        rpool = ctx.enter_context(tc.tile_pool(name="rpool", bufs=8))
        ypool = ctx.enter_context(tc.tile_pool(name="ypool", bufs=3))
        ps_pair = ctx.enter_context(tc.tile_pool(name="ps_pair", bufs=2, space="PSUM"))
        ps_work = ctx.enter_context(tc.tile_pool(name="ps_work", bufs=3, space="PSUM"))
        ps_tp = ctx.enter_context(tc.tile_pool(name="ps_tp", bufs=1, space="PSUM"))

        identity = persist.tile([P, P], bf16, tag="identity")
        make_identity(nc, identity)

        # HAM warmup + ACT table preload during the input-DMA wait: the PE
        # clock gate opens only after ~3.4us of sustained matmul activity,
        # and the first exp pays a ~2.7us activation-table DMA.  Burn both
        # on dummy work so the preamble runs at full clock.
        warm_sb = persist.tile([P, 8], f32, tag="warm")
        nc.vector.memset(warm_sb[:], 0.0)
        nc.scalar.activation(
            warm_sb[:, 0:1], warm_sb[:, 1:2], mybir.ActivationFunctionType.Exp
        )
        warm_ps = ps_tp.tile([P, 512], bf16, tag="tp", name="warm_ps")
        for _ in range(56):
            nc.tensor.transpose(warm_ps[:, :P], identity, identity)

        # ---- loads ----
        # DMA triggers serialize at ~725ns on the Sync engine, so use few,
        # wide strided descriptors and order them so only the K3+Q0 weight
        # columns and the first xT n-quarter gate the preamble chunks.
        wq_sb = persist.tile([P, CB, 3 * EG], bf16, tag="wq")
        wqv = wqkv[:].rearrange("(cb p) e -> p cb e", p=P)
        xts_sb = persist.tile([P, CB, N], bf16, tag="xt")
        xtv = xT[:].rearrange("(cb p) n -> p cb n", p=P)
        nc.sync.dma_start(wq_sb[:, :, 3 * P : 4 * P], wqv[:, :, 3 * P : 4 * P])  # K3
        nc.sync.dma_start(wq_sb[:, :, 0:P], wqv[:, :, 0:P])                      # Q0
        nc.sync.dma_start(xts_sb[:, :, 0:512], xtv[:, :, 0:512])
        nc.sync.dma_start(xts_sb[:, :, 512:2048], xtv[:, :, 512:2048])
        nc.sync.dma_start(wq_sb[:, :, 2 * EG :], wqv[:, :, 2 * EG :])            # V
        nc.sync.dma_start(wq_sb[:, :, 4 * P : 6 * P], wqv[:, :, 4 * P : 6 * P])  # K4,K5
        nc.sync.dma_start(wq_sb[:, :, P : 3 * P], wqv[:, :, P : 3 * P])          # Q1,Q2
        xts = [xts_sb[:, cb] for cb in range(CB)]
        wp_sb = persist.tile([P, EG // P, C], bf16, tag="wp")
        nc.sync.dma_start(wp_sb[:], wproj[:].rearrange("(cb p) c -> p cb c", p=P))

        qkT_sb = persist.tile([P, 2 * EG // P, N], bf16, tag="qkT")
        vp_sb = persist.tile([P, NT, HL * (D + 1)], bf16, tag="vp")
        vp4 = vp_sb.rearrange("p m (h c) -> p m h c", c=D + 1)
        nc.vector.memset(vp4[:, :, :, D : D + 1], 1.0)
        og_sb = persist.tile([P, NT, EG], bf16, tag="og")   # heads out [n, ch]

        def qk_chunk(eb, nch):
            def go():
                qpsum = ps_work.tile([P, 512], f32, tag="w", name="qpsum")
                for cb in range(CB):
                    nc.tensor.matmul(
                        qpsum,
                        wq_sb[:, cb, eb * P : (eb + 1) * P],
                        xts[cb][:, nch * 512 : (nch + 1) * 512],
                        start=(cb == 0),
                        stop=(cb == CB - 1),
                    )
                nc.vector.tensor_copy(
                    qkT_sb[:, eb, nch * 512 : (nch + 1) * 512], qpsum
                )
            return go

        def v_group(mt):
            def go():
                vpsum = ps_work.tile([P, 512], f32, tag="w", name="vpsum")
                for cb in range(CB):
                    nc.tensor.matmul(
                        vpsum[:, :EG],
                        xts[cb][:, mt * P : (mt + 1) * P],
                        wq_sb[:, cb, 2 * EG : 3 * EG],
                        start=(cb == 0),
                        stop=(cb == CB - 1),
                    )
                nc.vector.tensor_copy(
                    vp4[:, mt, :, :D],
                    vpsum[:, :EG].rearrange("p (h d) -> p h d", d=D),
                )
            return go

        # pts tiles: pts_q[j][q][t] = [128, mt-parity, head-slot, 512] bf16
        pts_q = [[None] * 4 for _ in range(NP)]

        def emit_pair_quarter(j, q, work=None, first_mts=0):
            """Scores+exp for head pair j (heads 2j, 2j+1), n-quarter q.
            `work` closures spread across the 16 mt slots.  `first_mts`:
            emit only mt < first_mts (preamble partial) or mt >= first_mts
            (rest) when splitting; 0 means all 16 here."""
            kblk, qblk = 3 + j, j
            if pts_q[j][q] is None:
                pts_q[j][q] = [
                    ptpool.tile([P, 2, 2, 512], bf16, tag="pt", bufs=PT_BUFS,
                                name=f"pts{j}_{q}_{t}")
                    for t in range(8)
                ]
            tiles = pts_q[j][q]
            work = work or []
            wi = 0
            for mt in range(16):
                psp = ps_pair.tile([P, 1024], f32, tag="pair", name="psp")
                for s in range(2):
                    prow = s * D
                    nc.tensor.matmul(
                        psp[:, s * 512 : (s + 1) * 512],
                        qkT_sb[prow : prow + D, kblk, mt * P : (mt + 1) * P],
                        qkT_sb[prow : prow + D, qblk, q * 512 : (q + 1) * 512],
                        start=True,
                        stop=True,
                    )
                nc.scalar.activation(
                    tiles[mt // 2][:, mt % 2],
                    psp,
                    mybir.ActivationFunctionType.Exp,
                    scale=SCALE,
                )
                hi = (mt + 1) * len(work) // 16
                while wi < hi:
                    work[wi]()
                    wi += 1
            while wi < len(work):
                work[wi]()
                wi += 1

        def pv_chain(j, s, nt):
            """PV + normalize for global head 2j+s, row tile nt."""
            def go():
                q, r = nt // 4, nt % 4
                tiles = pts_q[j][q]
                h = 2 * j + s
                pvpsum = ps_work.tile([P, 512], f32, tag="w", name="pvpsum")
                for mt in range(NT):
                    nc.tensor.matmul(
                        pvpsum[:, : D + 1],
                        tiles[mt // 2][:, mt % 2, s, r * P : (r + 1) * P],
                        vp_sb[:, mt, h * (D + 1) : (h + 1) * (D + 1)],
                        start=(mt == 0),
                        stop=(mt == NT - 1),
                    )
                rr = rpool.tile([P, 1], f32, tag="r", name="r")
                nc.vector.reciprocal(rr, pvpsum[:, D : D + 1])
                nc.vector.tensor_scalar(
                    og_sb[:, nt, h * D : (h + 1) * D],
                    pvpsum[:, :D],
                    rr,
                    None,
                    mybir.AluOpType.mult,
                )
            return go

        ogTs = [ptpool.tile([P, N], bf16, tag="ogT", bufs=EG // P, name=f"ogT{cb}")
                for cb in range(EG // P)]

        def transpose_one(cb, nt):
            def go():
                tpsum = ps_tp.tile([P, 512], bf16, tag="tp", name="tpsum")
                nc.tensor.transpose(
                    tpsum[:, :P], og_sb[:, nt, cb * P : (cb + 1) * P], identity
                )
                nc.vector.tensor_copy(
                    ogTs[cb][:, nt * P : (nt + 1) * P], tpsum[:, :P]
                )
            return go

        yv = y[:].rearrange("(nt p) c -> p nt c", p=P)

        def proj_one(nt):
            def go():
                y_sb = ypool.tile([P, C], bf16, tag="y", name="y_sb")
                for half in range(2):
                    ppsum = ps_work.tile([P, 512], f32, tag="w", name="ppsum")
                    for cb in range(EG // P):
                        nc.tensor.matmul(
                            ppsum[:, :EG],
                            ogTs[cb][:, nt * P : (nt + 1) * P],
                            wp_sb[:, cb, half * EG : (half + 1) * EG],
                            start=(cb == 0),
                            stop=(cb == EG // P - 1),
                        )
                    nc.vector.tensor_copy(
                        y_sb[:, half * EG : (half + 1) * EG], ppsum[:, :EG]
                    )
                nc.sync.dma_start(yv[:, nt], y_sb)
            return go

        def pv4(j, q2):
            """The 8 pv chains for pair j, quarter q2 (both heads)."""
            return [pv_chain(j, s, 4 * q2 + r) for r in (0, 1) for s in (0, 1)] + \
                   [pv_chain(j, s, 4 * q2 + r) for r in (2, 3) for s in (0, 1)]

        def tp4(j, q2):
            return [transpose_one(j, 4 * q2 + r) for r in range(4)]

        # ---- schedule ----
        # Preamble: K3 chunk 0 + Q0 chunk 0 (gates pair-0 quarter-0 mts 0-3).
        qk_chunk(3, 0)()
        qk_chunk(0, 0)()

        W = {
            (0, 0): [qk_chunk(3, 1), v_group(0), v_group(1),
                     qk_chunk(3, 2), v_group(2), v_group(3),
                     qk_chunk(3, 3), v_group(4), v_group(5), qk_chunk(0, 1)],
            (0, 1): [v_group(6), v_group(7), v_group(8), v_group(9),
                     qk_chunk(0, 2), v_group(10), v_group(11), v_group(12),
                     v_group(13), v_group(14), v_group(15)],
            (0, 2): [qk_chunk(0, 3), qk_chunk(4, 0)] + pv4(0, 0)
                    + [qk_chunk(4, 1)],
            (0, 3): [qk_chunk(4, 2)] + pv4(0, 1) + [qk_chunk(4, 3), qk_chunk(1, 0)],
            (1, 0): [qk_chunk(1, 1)] + pv4(0, 2) + tp4(0, 0),
            (1, 1): [qk_chunk(1, 2)] + pv4(0, 3) + tp4(0, 1) + [qk_chunk(5, 0)],
            (1, 2): [qk_chunk(1, 3), qk_chunk(5, 1)] + pv4(1, 0) + tp4(0, 2),
            (1, 3): [qk_chunk(5, 2)] + pv4(1, 1) + tp4(0, 3)
                    + [qk_chunk(5, 3), qk_chunk(2, 0)],
            (2, 0): [qk_chunk(2, 1)] + pv4(1, 2) + tp4(1, 0) + tp4(1, 1),
            (2, 1): [qk_chunk(2, 2)] + pv4(1, 3) + pv4(2, 0) + tp4(2, 0),
            (2, 2): [qk_chunk(2, 3)] + pv4(2, 1) + tp4(1, 2) + tp4(2, 1)
                    + [proj_one(0), proj_one(1)],
            (2, 3): tp4(1, 3) + pv4(2, 2) + tp4(2, 2)
                    + [proj_one(2), proj_one(3), proj_one(4), proj_one(5)],
        }
        for j in range(NP):
            for q in range(4):
                emit_pair_quarter(j, q, work=W[(j, q)])

        # ---- tail ----
        # Per-nt chains (pv both heads -> transpose -> proj -> dma) keep the
        # output DMAs flowing instead of piling up after the last proj.
        tail = []
        fill = [proj_one(6), proj_one(7)] + [proj_one(nt) for nt in range(8, 12)]
        for i, nt in enumerate(range(12, 16)):
            tail += [pv_chain(2, 0, nt), pv_chain(2, 1, nt), transpose_one(2, nt),
                     proj_one(nt)]
            tail += fill[i * 2 : i * 2 + 2]
        tail += fill[8:]
        for w in tail:
            w()

    nc.compile()
    return nc


_PROGRAM = None


def _get_program():
    global _PROGRAM
    if _PROGRAM is None:
        _PROGRAM = _build_program()
    return _PROGRAM


def _shard_inputs(x, Wqkv, Wproj):
    bf = ml_dtypes.bfloat16
    in_maps = []
    for core in range(NCORES):
        b, g = core // G, core % G
        xT = np.ascontiguousarray(x[b].T).astype(bf)
        wg = np.concatenate(
            [
                Wqkv[:, g * EG : (g + 1) * EG],
                Wqkv[:, C + g * EG : C + (g + 1) * EG],
                Wqkv[:, 2 * C + g * EG : 2 * C + (g + 1) * EG],
            ],
            axis=1,
        ).astype(bf)
        wp = np.ascontiguousarray(Wproj[g * EG : (g + 1) * EG, :]).astype(bf)
        in_maps.append({"xT": xT, "wqkv": wg, "wproj": wp})
    return in_maps


def _run(x, Wqkv, Wproj, bproj, trace=False):
    nc = _get_program()
    in_maps = _shard_inputs(x, Wqkv, Wproj)
    res = run_bass_kernel_spmd(nc, in_maps, list(range(NCORES)), trace=trace)
    out = np.empty((B, N, C), np.float32)
    for b in range(B):
        out[b] = (
            res.results[b * G]["y"].astype(np.float32)
            + res.results[b * G + 1]["y"].astype(np.float32)
            + bproj
        )
    return out, res


def kernel(x, Wqkv, Wproj, bproj):
    x = np.asarray(x, np.float32)
    Wqkv = np.asarray(Wqkv, np.float32)
    Wproj = np.asarray(Wproj, np.float32)
    bproj = np.asarray(bproj, np.float32)
    out, _ = _run(x, Wqkv, Wproj, bproj)
    return out


# revision 19
# speedup vs baseline: 1.0280x; 1.0280x over previous
"""Multi-head attention block (QKV linear -> softmax attention -> proj linear)
for Trainium2, SPMD over 8 NeuronCores.

Sharding: 8 shards = batch (4) x head-group (2 groups of 6 heads).
Each core computes, for its (b, g):
    qkv   = x[b] @ Wqkv[:, cols(g)]            (bf16 matmul, fp32 accum)
    S^T_h = K_h Q_h^T   per head               (keys on partitions)
    P^T_h = exp(SCALE * S^T_h)                 (ACT engine, bf16 out)
    out_h = (P_h @ [V_h | 1]) -> normalize rows by the ones-column sum
    y_g   = concat_h(out_h) @ Wproj[rows(g), :]    (partial, fp32 out)
Host sums the two head-group partials per batch and adds bproj.

Head-PAIR emission: heads 2j and 2j+1 have their Q/K channels on SBUF
partitions 0-63 / 64-127 of the same qkT block, so their K=64 score
matmuls carry PE tile positions (0,0) / (64,0) and run CONCURRENTLY in
the PE array when issued back-to-back (measured 3x vs serial).  Scores
are emitted in n-quarters (512 cols); each (mt, quarter) fills a
[128,1024] psum pair-chunk ([head 2j | head 2j+1]) consumed by one
1024-wide exp ACTIVATE into a packed pts tile [128, mt-parity, head,
512].  PV/transposes/proj interleave as work items one quarter behind.

Shapes hardcoded: x [4, 2048, 768], Wqkv [768, 2304], Wproj [768, 768].
"""

import os
from contextlib import ExitStack

import numpy as np
import ml_dtypes

import concourse.bass as bass
import concourse.mybir as mybir
import concourse.tile as tile
from concourse import bacc
from concourse.bass_utils import run_bass_kernel_spmd
from concourse.masks import make_identity

B, N, C = 4, 2048, 768
H, D = 12, 64          # total heads, head dim
G = 2                  # head groups (tensor-parallel axis)
HL = H // G            # heads per core = 6
NP = HL // 2           # head pairs per core = 3
SCALE = D ** -0.5
P = 128
CB = C // P            # 6 contraction blocks
NT = N // P            # 16 row tiles
EG = HL * D            # 384 = per-group width of Q / K / V
NCORES = 8

f32 = mybir.dt.float32
bf16 = mybir.dt.bfloat16

PT_BUFS = int(os.environ.get("KRN_PT_BUFS", "24"))


def _build_program():
    nc = bacc.Bacc("TRN2", target_bir_lowering=False, debug=False)

    xT = nc.dram_tensor("xT", [C, N], bf16, kind="ExternalInput")           # x[b].T
    wqkv = nc.dram_tensor("wqkv", [C, 3 * EG], bf16, kind="ExternalInput")  # [Qg|Kg|Vg]
    wproj = nc.dram_tensor("wproj", [EG, C], bf16, kind="ExternalInput")    # group rows
    y = nc.dram_tensor("y", [N, C], bf16, kind="ExternalOutput")            # partial out

    with tile.TileContext(nc) as tc, ExitStack() as ctx:
        persist = ctx.enter_context(tc.tile_pool(name="persist", bufs=1))
        ptpool = ctx.enter_context(tc.tile_pool(name="ptpool", bufs=1))
        rpool = ctx.enter_context(tc.tile_pool(name="rpool", bufs=8))
        ypool = ctx.enter_context(tc.tile_pool(name="ypool", bufs=3))
        ps_pair = ctx.enter_context(tc.tile_pool(name="ps_pair", bufs=2, space="PSUM"))
        ps_work = ctx.enter_context(tc.tile_pool(name="ps_work", bufs=3, space="PSUM"))
        ps_tp = ctx.enter_context(tc.tile_pool(name="ps_tp", bufs=1, space="PSUM"))

        identity = persist.tile([P, P], bf16, tag="identity")
        make_identity(nc, identity)

        # HAM warmup + ACT table preload during the input-DMA wait: the PE
        # clock gate opens only after ~3.4us of sustained matmul activity,
        # and the first exp pays a ~2.7us activation-table DMA.  Burn both
        # on dummy work so the preamble runs at full clock.
        warm_sb = persist.tile([P, 8], f32, tag="warm")
        nc.vector.memset(warm_sb[:], 0.0)
        nc.scalar.activation(
            warm_sb[:, 0:1], warm_sb[:, 1:2], mybir.ActivationFunctionType.Exp
        )
        warm_ps = ps_tp.tile([P, 512], bf16, tag="tp", name="warm_ps")
        for _ in range(56):
            nc.tensor.transpose(warm_ps[:, :P], identity, identity)

        # ---- loads ----
        # DMA triggers serialize at ~725ns on the Sync engine, so use few,
        # wide strided descriptors and order them so only the K3+Q0 weight
        # columns and the first xT n-quarter gate the preamble chunks.
        wq_sb = persist.tile([P, CB, 3 * EG], bf16, tag="wq")
        wqv = wqkv[:].rearrange("(cb p) e -> p cb e", p=P)
        xts_sb = persist.tile([P, CB, N], bf16, tag="xt")
        xtv = xT[:].rearrange("(cb p) n -> p cb n", p=P)
        nc.sync.dma_start(wq_sb[:, :, 3 * P : 4 * P], wqv[:, :, 3 * P : 4 * P])  # K3
        nc.sync.dma_start(wq_sb[:, :, 0:P], wqv[:, :, 0:P])                      # Q0
        nc.sync.dma_start(xts_sb[:, :, 0:512], xtv[:, :, 0:512])
        nc.sync.dma_start(wq_sb[:, :, 2 * EG :], wqv[:, :, 2 * EG :])            # V
        nc.sync.dma_start(xts_sb[:, :, 512:1024], xtv[:, :, 512:1024])
        nc.sync.dma_start(xts_sb[:, :, 1024:2048], xtv[:, :, 1024:2048])
        nc.sync.dma_start(wq_sb[:, :, 4 * P : 6 * P], wqv[:, :, 4 * P : 6 * P])  # K4,K5
        nc.sync.dma_start(wq_sb[:, :, P : 3 * P], wqv[:, :, P : 3 * P])          # Q1,Q2
        xts = [xts_sb[:, cb] for cb in range(CB)]
        wp_sb = persist.tile([P, EG // P, C], bf16, tag="wp")
        nc.sync.dma_start(wp_sb[:], wproj[:].rearrange("(cb p) c -> p cb c", p=P))

        qkT_sb = persist.tile([P, 2 * EG // P, N], bf16, tag="qkT")
        vp_sb = persist.tile([P, NT, HL * (D + 1)], bf16, tag="vp")
        vp4 = vp_sb.rearrange("p m (h c) -> p m h c", c=D + 1)
        nc.vector.memset(vp4[:, :, :, D : D + 1], 1.0)
        og_sb = persist.tile([P, NT, EG], bf16, tag="og")   # heads out [n, ch]

        def qk_chunk(eb, nch):
            def go():
                qpsum = ps_work.tile([P, 512], f32, tag="w", name="qpsum")
                for cb in range(CB):
                    nc.tensor.matmul(
                        qpsum,
                        wq_sb[:, cb, eb * P : (eb + 1) * P],
                        xts[cb][:, nch * 512 : (nch + 1) * 512],
                        start=(cb == 0),
                        stop=(cb == CB - 1),
                    )
                nc.vector.tensor_copy(
                    qkT_sb[:, eb, nch * 512 : (nch + 1) * 512], qpsum
                )
            return go

        def v_group(mt):
            def go():
                vpsum = ps_work.tile([P, 512], f32, tag="w", name="vpsum")
                for cb in range(CB):
                    nc.tensor.matmul(
                        vpsum[:, :EG],
                        xts[cb][:, mt * P : (mt + 1) * P],
                        wq_sb[:, cb, 2 * EG : 3 * EG],
                        start=(cb == 0),
                        stop=(cb == CB - 1),
                    )
                nc.vector.tensor_copy(
                    vp4[:, mt, :, :D],
                    vpsum[:, :EG].rearrange("p (h d) -> p h d", d=D),
                )
            return go

        # pts tiles: pts_q[j][q][t] = [128, mt-parity, head-slot, 512] bf16
        pts_q = [[None] * 4 for _ in range(NP)]

        def emit_pair_quarter(j, q, work=None, first_mts=0):
            """Scores+exp for head pair j (heads 2j, 2j+1), n-quarter q.
            `work` closures spread across the 16 mt slots.  `first_mts`:
            emit only mt < first_mts (preamble partial) or mt >= first_mts
            (rest) when splitting; 0 means all 16 here."""
            kblk, qblk = 3 + j, j
            if pts_q[j][q] is None:
                pts_q[j][q] = [
                    ptpool.tile([P, 2, 2, 512], bf16, tag="pt", bufs=PT_BUFS,
                                name=f"pts{j}_{q}_{t}")
                    for t in range(8)
                ]
            tiles = pts_q[j][q]
            work = work or []
            wi = 0
            for mt in range(16):
                psp = ps_pair.tile([P, 1024], f32, tag="pair", name="psp")
                for s in range(2):
                    prow = s * D
                    nc.tensor.matmul(
                        psp[:, s * 512 : (s + 1) * 512],
                        qkT_sb[prow : prow + D, kblk, mt * P : (mt + 1) * P],
                        qkT_sb[prow : prow + D, qblk, q * 512 : (q + 1) * 512],
                        start=True,
                        stop=True,
                    )
                nc.scalar.activation(
                    tiles[mt // 2][:, mt % 2],
                    psp,
                    mybir.ActivationFunctionType.Exp,
                    scale=SCALE,
                )
                hi = (mt + 1) * len(work) // 16
                while wi < hi:
                    work[wi]()
                    wi += 1
            while wi < len(work):
                work[wi]()
                wi += 1

        def pv_chain(j, s, nt):
            """PV + normalize for global head 2j+s, row tile nt."""
            def go():
                q, r = nt // 4, nt % 4
                tiles = pts_q[j][q]
                h = 2 * j + s
                pvpsum = ps_work.tile([P, 512], f32, tag="w", name="pvpsum")
                for mt in range(NT):
                    nc.tensor.matmul(
                        pvpsum[:, : D + 1],
                        tiles[mt // 2][:, mt % 2, s, r * P : (r + 1) * P],
                        vp_sb[:, mt, h * (D + 1) : (h + 1) * (D + 1)],
                        start=(mt == 0),
                        stop=(mt == NT - 1),
                    )
                rr = rpool.tile([P, 1], f32, tag="r", name="r")
                nc.vector.reciprocal(rr, pvpsum[:, D : D + 1])
                nc.vector.tensor_scalar(
                    og_sb[:, nt, h * D : (h + 1) * D],
                    pvpsum[:, :D],
                    rr,
                    None,
                    mybir.AluOpType.mult,
                )
            return go

        ogTs = [ptpool.tile([P, N], bf16, tag="ogT", bufs=EG // P, name=f"ogT{cb}")
                for cb in range(EG // P)]

        def transpose_one(cb, nt):
            def go():
                tpsum = ps_tp.tile([P, 512], bf16, tag="tp", name="tpsum")
                nc.tensor.transpose(
                    tpsum[:, :P], og_sb[:, nt, cb * P : (cb + 1) * P], identity
                )
                nc.vector.tensor_copy(
                    ogTs[cb][:, nt * P : (nt + 1) * P], tpsum[:, :P]
                )
            return go

        yv = y[:].rearrange("(nt p) c -> p nt c", p=P)

        def proj_one(nt):
            def go():
                y_sb = ypool.tile([P, C], bf16, tag="y", name="y_sb")
                for half in range(2):
                    ppsum = ps_work.tile([P, 512], f32, tag="w", name="ppsum")
                    for cb in range(EG // P):
                        nc.tensor.matmul(
                            ppsum[:, :EG],
                            ogTs[cb][:, nt * P : (nt + 1) * P],
                            wp_sb[:, cb, half * EG : (half + 1) * EG],
                            start=(cb == 0),
                            stop=(cb == EG // P - 1),
                        )
                    nc.vector.tensor_copy(
                        y_sb[:, half * EG : (half + 1) * EG], ppsum[:, :EG]
                    )
                nc.sync.dma_start(yv[:, nt], y_sb)
            return go

        def pv4(j, q2):
            """The 8 pv chains for pair j, quarter q2 (both heads)."""
            return [pv_chain(j, s, 4 * q2 + r) for r in (0, 1) for s in (0, 1)] + \
                   [pv_chain(j, s, 4 * q2 + r) for r in (2, 3) for s in (0, 1)]

        def tp4(j, q2):
            return [transpose_one(j, 4 * q2 + r) for r in range(4)]

        # ---- schedule ----
        # Preamble: K3 chunk 0 + Q0 chunk 0 (gates pair-0 quarter-0 mts 0-3).
        qk_chunk(3, 0)()
        qk_chunk(0, 0)()

        W = {
            (0, 0): [qk_chunk(3, 1), v_group(0), v_group(1),
                     qk_chunk(3, 2), v_group(2), v_group(3),
                     qk_chunk(3, 3), v_group(4), v_group(5), qk_chunk(0, 1)],
            (0, 1): [v_group(6), v_group(7), v_group(8), v_group(9),
                     qk_chunk(0, 2), v_group(10), v_group(11), v_group(12),
                     v_group(13), v_group(14), v_group(15)],
            (0, 2): [qk_chunk(0, 3), qk_chunk(4, 0)] + pv4(0, 0)
                    + [qk_chunk(4, 1)],
            (0, 3): [qk_chunk(4, 2)] + pv4(0, 1) + [qk_chunk(4, 3), qk_chunk(1, 0)],
            (1, 0): [qk_chunk(1, 1)] + pv4(0, 2) + tp4(0, 0) + [qk_chunk(1, 2)],
            (1, 1): [qk_chunk(1, 3)] + pv4(0, 3) + pv4(1, 0) + tp4(0, 1),
            (1, 2): [qk_chunk(5, 0), qk_chunk(5, 1)] + pv4(1, 1)
                    + tp4(0, 2) + tp4(0, 3),
            (1, 3): [qk_chunk(5, 2), qk_chunk(5, 3)] + pv4(1, 2) + tp4(1, 0)
                    + [qk_chunk(2, 0)],
            (2, 0): [qk_chunk(2, 1)] + pv4(1, 3) + tp4(1, 1) + tp4(1, 2),
            (2, 1): [qk_chunk(2, 2)] + pv4(2, 0) + tp4(2, 0) + tp4(1, 3)
                    + [proj_one(0)],
            (2, 2): [qk_chunk(2, 3)] + pv4(2, 1) + tp4(2, 1)
                    + [proj_one(1), proj_one(2), proj_one(3)],
            (2, 3): pv4(2, 2) + tp4(2, 2)
                    + [proj_one(4), proj_one(5), proj_one(6), proj_one(7),
                       proj_one(8), proj_one(9)],
        }
        for j in range(NP):
            for q in range(4):
                emit_pair_quarter(j, q, work=W[(j, q)])

        # ---- tail ----
        # Per-nt chains (pv both heads -> transpose -> proj -> dma) keep the
        # output DMAs flowing instead of piling up after the last proj.
        tail = []
        fill = [proj_one(10), proj_one(11)]
        for i, nt in enumerate(range(12, 16)):
            tail += [pv_chain(2, 0, nt), pv_chain(2, 1, nt), transpose_one(2, nt),
                     proj_one(nt)]
            tail += fill[i : i + 1]
        for w in tail:
            w()

    nc.compile()
    return nc


_PROGRAM = None


def _get_program():
    global _PROGRAM
    if _PROGRAM is None:
        _PROGRAM = _build_program()
    return _PROGRAM


def _shard_inputs(x, Wqkv, Wproj):
    bf = ml_dtypes.bfloat16
    in_maps = []
    for core in range(NCORES):
        b, g = core // G, core % G
        xT = np.ascontiguousarray(x[b].T).astype(bf)
        wg = np.concatenate(
            [
                Wqkv[:, g * EG : (g + 1) * EG],
                Wqkv[:, C + g * EG : C + (g + 1) * EG],
                Wqkv[:, 2 * C + g * EG : 2 * C + (g + 1) * EG],
            ],
            axis=1,
        ).astype(bf)
        wp = np.ascontiguousarray(Wproj[g * EG : (g + 1) * EG, :]).astype(bf)
        in_maps.append({"xT": xT, "wqkv": wg, "wproj": wp})
    return in_maps


def _run(x, Wqkv, Wproj, bproj, trace=False):
    nc = _get_program()
    in_maps = _shard_inputs(x, Wqkv, Wproj)
    res = run_bass_kernel_spmd(nc, in_maps, list(range(NCORES)), trace=trace)
    out = np.empty((B, N, C), np.float32)
    for b in range(B):
        out[b] = (
            res.results[b * G]["y"].astype(np.float32)
            + res.results[b * G + 1]["y"].astype(np.float32)
            + bproj
        )
    return out, res


def kernel(x, Wqkv, Wproj, bproj):
    x = np.asarray(x, np.float32)
    Wqkv = np.asarray(Wqkv, np.float32)
    Wproj = np.asarray(Wproj, np.float32)
    bproj = np.asarray(bproj, np.float32)
    out, _ = _run(x, Wqkv, Wproj, bproj)
    return out
